# revision 21
# baseline (speedup 1.0000x reference)
"""H-sharded LSTM + FC + log_softmax on 8 trn2 cores.

Reference: T=1024 steps of LSTM(I=512 -> H=1024) over batch B=128, then
FC(H -> A=256) + log_softmax per step. Output [T, B, A] fp32.

Sharding: core k owns hidden chunk k (128 units) = 512 gate columns
(reordered [i,f,o,g]). Per step, cores exchange their 128x128 hT chunk
via per-slot remote_dma_broadcast (XOR-relative dests). Every core
computes the full FC + log_softmax (PE cost is row-bound, not
batch-bound) but only outputs its own 32 A-columns (column-permuted so
they are first); host concatenates.

pi = logical->physical tpb mapping, measured on this box via a tag
exchange: [0,1,2,3,6,7,4,5] (involution). Receiver r's slot j holds the
chunk of logical core sigma_r(j) = pi^-1(pi(r)^j); weights are permuted
host-side to match, so one SPMD NEFF works for all cores.
"""

import numpy as np

T_FULL = 1024
B = 128
I = 512
H = 1024
A = 256
NC = 8
PI = [0, 1, 2, 3, 6, 7, 4, 5]

_CACHE = {}


def _build(T):
    from concourse import bass, library_config

    mybir = bass.mybir
    f32 = mybir.dt.float32
    f32r = mybir.dt.float32r
    Sig = mybir.ActivationFunctionType.Sigmoid
    Tanh = mybir.ActivationFunctionType.Tanh
    Exp = mybir.ActivationFunctionType.Exp
    Ln = mybir.ActivationFunctionType.Ln

    nc = bass.Bass("TRN2", target_bir_lowering=False, debug=False, num_devices=NC)

    x_d = nc.dram_tensor("x", [T * 128, 512], f32r, kind="ExternalInput")
    wih_d = nc.dram_tensor("w_ih", [128, 2048], f32r, kind="ExternalInput")
    whh_d = nc.dram_tensor("w_hh", [128, 4096], f32r, kind="ExternalInput")
    wfc_d = nc.dram_tensor("w_fc", [128, 2048], f32r, kind="ExternalInput")
    b_d = nc.dram_tensor("b", [1, 512], f32, kind="ExternalInput")
    bfc_d = nc.dram_tensor("b_fc", [1, 256], f32, kind="ExternalInput")
    eye_d = nc.dram_tensor("eye", [128, 128], f32, kind="ExternalInput")
    ones_d = nc.dram_tensor("ones", [1, 128], f32, kind="ExternalInput")
    out_d = nc.dram_tensor("out", [T * 128, 32], f32, kind="ExternalOutput")

    sb = nc.alloc_sbuf_tensor
    w_ih_sb = sb("w_ih_sb", [128, 2048], f32r)
    w_hh_sb = sb("w_hh_sb", [128, 4096], f32r)
    w_fc_sb = sb("w_fc_sb", [128, 2048], f32r)
    bias_sb = sb("bias_sb", [1, 512], f32)
    bfc_sb = sb("bfc_sb", [1, 256], f32)
    eye_sb = sb("eye_sb", [128, 128], f32)
    ones_sb = sb("ones_sb", [1, 128], f32)
    x_sb = [sb(f"x_sb{i}", [128, 512], f32r) for i in range(3)]
    hT_own = [sb(f"hT_own{i}", [128, 128], f32r) for i in range(2)]
    slots = [
        [None] + [sb(f"slot{p}_{j}", [128, 128], f32r) for j in range(1, 8)]
        for p in range(2)
    ]
    h_new = sb("h_new", [128, 128], f32)
    c_sb = sb("c_sb", [128, 128], f32)
    ig_sb = sb("ig_sb", [128, 128], f32)
    tc_sb = sb("tc_sb", [128, 128], f32)
    sig_sb = [sb(f"sig_sb{i}", [128, 384], f32) for i in range(2)]
    tg_sb = [sb(f"tg_sb{i}", [128, 128], f32) for i in range(2)]
    shifted = [sb(f"shifted{i}", [128, 256], f32) for i in range(2)]
    exp_sb = sb("exp_sb", [128, 256], f32)
    m_sb = sb("m_sb", [128, 1], f32)
    ssum_sb = sb("ssum_sb", [128, 1], f32)
    ls_sb = sb("ls_sb", [128, 1], f32)
    out_sb = [sb(f"out_sb{i}", [128, 32], f32) for i in range(2)]

    gates_ps = [nc.alloc_psum_tensor(f"gates_ps{i}", [128, 512], f32) for i in range(2)]
    fc_ps = [nc.alloc_psum_tensor(f"fc_ps{i}", [128, 256], f32) for i in range(2)]
    tr_ps = [nc.alloc_psum_tensor(f"tr_ps{i}", [128, 128], f32) for i in range(2)]

    sem_w = nc.alloc_semaphore("sem_w")
    sem_xs = [nc.alloc_semaphore(f"sem_x{i}") for i in range(3)]
    sem_mmg = nc.alloc_semaphore("sem_mmg")
    sem_fc = nc.alloc_semaphore("sem_fc")
    sem_g = nc.alloc_semaphore("sem_g")
    sem_dg = nc.alloc_semaphore("sem_dg")
    sem_tc = nc.alloc_semaphore("sem_tc")
    sem_tr = nc.alloc_semaphore("sem_tr")
    sem_hT = nc.alloc_semaphore("sem_hT")
    sem_sendp = [nc.alloc_semaphore(f"sem_send{p}") for p in range(2)]
    sem_prep = nc.alloc_semaphore("sem_prep")
    sem_slot = [None] + [
        [nc.alloc_semaphore(f"sem_slot{j}_{p}") for p in range(2)]
        for j in range(1, 8)
    ]
    sem_sm = nc.alloc_semaphore("sem_sm")
    sem_exp = nc.alloc_semaphore("sem_exp")
    sem_ln = nc.alloc_semaphore("sem_ln")
    sem_fin = nc.alloc_semaphore("sem_fin")
    sem_outp = [nc.alloc_semaphore(f"sem_out{p}") for p in range(2)]

    # cumulative gate-MM count after step t completes (t=0: 5, else 13/step)
    cumG = [5 + 13 * t for t in range(T)]
    # cumulative sem_dg after the c-update / after the h-mul of step t
    dg_c = [1 if t == 0 else 4 * t + 1 for t in range(T)]
    dg_h = [2 if t == 0 else 4 * t + 2 for t in range(T)]

    def r(ap):
        return ap

    with nc.Block() as block:

        @block.sync
        def _(sync):
            for dst, src in [
                (w_ih_sb, wih_d),
                (w_hh_sb, whh_d),
                (w_fc_sb, wfc_d),
                (bias_sb, b_d),
                (bfc_sb, bfc_d),
                (eye_sb, eye_d),
                (ones_sb, ones_d),
            ]:
                sync.dma_start(out=dst[:], in_=src[:]).then_inc(sem_w, 16)
            for t in range(min(2, T)):
                sync.dma_start(
                    out=x_sb[t % 3][:], in_=x_d[128 * t : 128 * (t + 1), :]
                ).then_inc(sem_xs[t % 3], 16)
            for t in range(T + 1):
                if t + 2 <= T - 1:
                    if t >= 1:
                        sync.wait_ge(sem_mmg, cumG[t - 1])
                    sync.dma_start(
                        out=x_sb[(t + 2) % 3][:],
                        in_=x_d[128 * (t + 2) : 128 * (t + 3), :],
                    ).then_inc(sem_xs[(t + 2) % 3], 16)
                if t >= 1:
                    sync.wait_ge(sem_fin, t)
                    sync.dma_start(
                        out=out_d[128 * (t - 1) : 128 * t, :],
                        in_=out_sb[(t - 1) % 2][:],
                    ).then_inc(sem_outp[(t - 1) % 2], 16)
            sync.wait_ge(sem_outp[0], 16 * ((T + 1) // 2))
            if T >= 2:
                sync.wait_ge(sem_outp[1], 16 * (T // 2))

        @block.tensor
        def _(tensor):
            mm = tensor.matmul
            tensor.wait_ge(sem_w, 112)
            for t in range(T):
                par = t % 2
                par1 = (t + 1) % 2
                # ---- gates into gates_ps[par] ----
                if t >= 2:
                    tensor.wait_ge(sem_g, 2 * (t - 1))
                tensor.wait_ge(sem_xs[t % 3], 16 * (t // 3 + 1))
                mm(
                    gates_ps[par][:], ones_sb[:], bias_sb[:], start=True, stop=False
                ).then_inc(sem_mmg, 1)
                for ci in range(4):
                    mm(
                        gates_ps[par][:],
                        r(x_sb[t % 3][:, 128 * ci : 128 * (ci + 1)]),
                        r(w_ih_sb[:, 512 * ci : 512 * (ci + 1)]),
                        start=False,
                        stop=(t == 0 and ci == 3),
                    ).then_inc(sem_mmg, 1)
                if t >= 1:
                    mm(
                        gates_ps[par][:],
                        r(hT_own[par][:]),
                        r(w_hh_sb[:, 0:512]),
                        start=False,
                        stop=False,
                    ).then_inc(sem_mmg, 1)
                    for j in range(1, 8):
                        mm(
                            gates_ps[par][:],
                            r(slots[par][j][:]),
                            r(w_hh_sb[:, 512 * j : 512 * (j + 1)]),
                            start=False,
                            stop=(j == 7),
                        ).then_inc(sem_mmg, 1)
                # ---- transpose h_{t+1} ----
                tensor.wait_ge(sem_dg, dg_h[t])
                tensor.transpose(tr_ps[par1][:], h_new[:], eye_sb[:]).then_inc(
                    sem_tr, 1
                )
                # ---- FC into fc_ps[par] ----
                if t >= 2:
                    tensor.wait_ge(sem_sm, 2 * (t - 1))
                mm(
                    fc_ps[par][:], ones_sb[:], bfc_sb[:], start=True, stop=False
                ).then_inc(sem_fc, 1)
                tensor.wait_ge(sem_hT, t + 1)
                mm(
                    fc_ps[par][:],
                    r(hT_own[par1][:]),
                    r(w_fc_sb[:, 0:256]),
                    start=False,
                    stop=False,
                ).then_inc(sem_fc, 1)
                for j in range(1, 8):
                    tensor.wait_ge(sem_slot[j][(t + 1) % 2], 2 * (t // 2 + 1))
                    mm(
                        fc_ps[par][:],
                        r(slots[par1][j][:]),
                        r(w_fc_sb[:, 256 * j : 256 * (j + 1)]),
                        start=False,
                        stop=(j == 7),
                    ).then_inc(sem_fc, 1)

        @block.scalar
        def _(scalar):
            for t in range(T + 1):
                if t >= 1:
                    # softmax pieces for out_{t-1}
                    scalar.wait_ge(sem_sm, 2 * t)
                    if t >= 2:
                        scalar.wait_ge(sem_fin, t - 1)
                    scalar.activation(
                        exp_sb[:], shifted[(t - 1) % 2][:], Exp,
                        accum_out=ssum_sb[:, 0:1],
                    ).then_inc(sem_exp, 1)
                    # accum_out drains late; order the Ln read behind it
                    scalar.wait_ge(sem_exp, t)
                    scalar.activation(ls_sb[:, 0:1], ssum_sb[:, 0:1], Ln).then_inc(
                        sem_ln, 1
                    )
                if t <= T - 1:
                    par = t % 2
                    scalar.wait_ge(sem_mmg, cumG[t])
                    if t >= 2:
                        scalar.wait_ge(sem_dg, dg_h[t - 2])
                    scalar.activation(
                        sig_sb[par][:], gates_ps[par][:, 0:384], Sig
                    ).then_inc(sem_g, 1)
                    scalar.activation(
                        tg_sb[par][:], gates_ps[par][:, 384:512], Tanh
                    ).then_inc(sem_g, 1)
                    scalar.wait_ge(sem_dg, dg_c[t])
                    scalar.activation(tc_sb[:], c_sb[:], Tanh).then_inc(sem_tc, 1)

        @block.vector
        def _(vector):
            for t in range(T + 1):
                if t >= 1:
                    # softmax for out_{t-1} over fc_ps[(t-1)%2]
                    pv = (t - 1) % 2
                    vector.wait_ge(sem_fc, 9 * t)
                    if t >= 2:
                        vector.wait_ge(sem_sm, 2 * (t - 1))
                    vector.tensor_reduce(
                        m_sb[:, 0:1],
                        fc_ps[pv][:],
                        axis=bass.mybir.AxisListType.X,
                        op=bass.mybir.AluOpType.max,
                    ).then_inc(sem_sm, 1)
                    if t >= 3:
                        vector.wait_ge(sem_exp, t - 2)
                    # TensorScalarPtr prefetches the scalar operand; order it
                    # after our own reduce via its sem inc
                    vector.wait_ge(sem_sm, 2 * t - 1)
                    vector.tensor_scalar_sub(
                        shifted[pv][:], fc_ps[pv][:], m_sb[:, 0:1]
                    ).then_inc(sem_sm, 1)
                    vector.wait_ge(sem_ln, t)
                    if t >= 3:
                        vector.wait_ge(sem_outp[(t - 1) % 2], 16 * ((t - 3) // 2 + 1))
                    vector.tensor_scalar_sub(
                        out_sb[pv][:], shifted[pv][:, 0:32], ls_sb[:, 0:1]
                    ).then_inc(sem_fin, 1)
                if t <= T - 1:
                    par = t % 2
                    vector.wait_ge(sem_g, 2 * (t + 1))
                    if t >= 1:
                        vector.wait_ge(sem_tc, t)
                        vector.wait_ge(sem_dg, 4 * t - 3)
                        vector.tensor_mul(
                            ig_sb[:], sig_sb[par][:, 0:128], tg_sb[par][:]
                        ).then_inc(sem_dg, 1)
                        vector.tensor_mul(
                            c_sb[:], sig_sb[par][:, 128:256], c_sb[:]
                        ).then_inc(sem_dg, 1)
                        vector.wait_ge(sem_dg, 4 * t)
                        vector.tensor_add(c_sb[:], c_sb[:], ig_sb[:]).then_inc(
                            sem_dg, 1
                        )
                    else:
                        vector.tensor_mul(
                            c_sb[:], sig_sb[0][:, 0:128], tg_sb[0][:]
                        ).then_inc(sem_dg, 1)
                    vector.wait_ge(sem_tc, t + 1)
                    if t >= 1:
                        vector.wait_ge(sem_tr, t)
                    vector.tensor_mul(
                        h_new[:], sig_sb[par][:, 256:384], tc_sb[:]
                    ).then_inc(sem_dg, 1)
                    # copy h^T out of PSUM for the exchange DMA + next-step MMs
                    if t >= 2:
                        vector.wait_ge(sem_sendp[(t + 1) % 2], 112 * ((t - 2) // 2 + 1))
                    vector.wait_ge(sem_tr, t + 1)
                    vector.tensor_copy(
                        hT_own[(t + 1) % 2][:], tr_ps[(t + 1) % 2][:]
                    ).then_inc(sem_hT, 1)

        @block.gpsimd
        def _(gp):
            gp.load_library(library_config.remote_dma)
            for t in range(T):
                par1 = (t + 1) % 2
                for j in range(1, 8):
                    rdests = [None] * 8
                    rdests[j] = (0, j)
                    gp.remote_dma_broadcast(
                        slots[par1][j][:],
                        hT_own[par1][:],
                        remote_sem=sem_slot[j][par1],
                        local_sem=sem_sendp[par1],
                        rdests=rdests,
                    ).then_inc(sem_prep, 1)
                gp.wait_ge(sem_prep, 7 * (t + 1))
                gp.wait_ge(sem_hT, t + 1)
                gp.trigger_dma(count=7)

    return nc


def _prep_inputs(x, W_ih, W_hh, b_ih, b_hh, W_fc, b_fc, T):
    f = np.float32
    # x: [T, B, I] -> [T*128, 512]: row t*128+p, col 128*ci+b = x[t, b, 128*ci+p]
    xT = np.ascontiguousarray(x[:T].transpose(0, 2, 1))  # [T, I, B]
    x_host = np.ascontiguousarray(
        xT.reshape(T, 4, 128, B).transpose(0, 2, 1, 3).reshape(T * 128, 512)
    ).astype(f, copy=False)
    eye = np.eye(128, dtype=f)
    ones = np.ones((1, 128), dtype=f)
    bsum = (b_ih + b_hh).astype(f)

    in_maps = []
    for k in range(NC):
        col_idx = np.concatenate(
            [
                np.arange(128 * k, 128 * k + 128),  # i
                1024 + np.arange(128 * k, 128 * k + 128),  # f
                3072 + np.arange(128 * k, 128 * k + 128),  # o
                2048 + np.arange(128 * k, 128 * k + 128),  # g
            ]
        )
        sigma = [PI[PI[k] ^ j] for j in range(8)]  # PI is an involution
        wih_T = W_ih[col_idx].T.astype(f)  # [512, 512]
        wih_host = np.ascontiguousarray(
            wih_T.reshape(4, 128, 512).transpose(1, 0, 2).reshape(128, 2048)
        )
        Whh_k = W_hh[col_idx].astype(f)  # [512, 1024]
        whh_host = np.concatenate(
            [Whh_k[:, 128 * s : 128 * (s + 1)].T for s in sigma], axis=1
        )  # [128, 4096]
        own_cols = np.arange(32 * k, 32 * k + 32)
        p_k = np.concatenate([own_cols, np.delete(np.arange(256), own_cols)])
        Wfc_p = W_fc[p_k].astype(f)  # [256, 1024]
        wfc_host = np.concatenate(
            [Wfc_p[:, 128 * s : 128 * (s + 1)].T for s in sigma], axis=1
        )  # [128, 2048]
        in_maps.append(
            {
                "x": x_host,
                "w_ih": np.ascontiguousarray(wih_host),
                "w_hh": np.ascontiguousarray(whh_host),
                "w_fc": np.ascontiguousarray(wfc_host),
                "b": bsum[col_idx].reshape(1, 512),
                "b_fc": b_fc.astype(f)[p_k].reshape(1, 256),
                "eye": eye,
                "ones": ones,
            }
        )
    return in_maps


def _run_hw(nc, in_maps):
    from concourse.bass_utils import run_bass_kernel_spmd
    from concourse.library_overlay import lower_extended_insts

    lower_extended_insts(nc)
    res = run_bass_kernel_spmd(nc, in_maps, list(range(NC)))
    return [res.results[k]["out"] for k in range(NC)]


def kernel(x, W_ih, W_hh, b_ih, b_hh, W_fc, b_fc, T=None):
    T = T or T_FULL
    in_maps = _prep_inputs(x, W_ih, W_hh, b_ih, b_hh, W_fc, b_fc, T)
    if T not in _CACHE:
        _CACHE[T] = _build(T)
    outs = _run_hw(_CACHE[T], in_maps)
    full = np.empty((T, B, A), dtype=np.float32)
    for k in range(NC):
        full[:, :, 32 * k : 32 * (k + 1)] = np.asarray(outs[k]).reshape(T, 128, 32)
    return full
